# revision 5
# baseline (speedup 1.0000x reference)
"""Causal self-attention (B=4, T=2048, C=1024, H=16) on 8 TRN2 NeuronCores.

Sharding: core = (batch b, head-group g); 4 batches x 2 groups of 8 heads.
Each core computes QKV for its 8 heads on its batch, causal attention, and
a partial projection output [T, C] (sum over its heads' channels). The host
sums the two group-partials per batch and adds b_proj.

v2 layout choices (per core):
  - x arrives pre-transposed from the host as xT [C, T] so the QKV matmuls
    contract along the partition dim with no on-chip transposes.
  - qT/kT are produced in [channel, T] layout, v in [T, channel] layout.
  - Scores are computed transposed: S^T[k, q] = lhsT(kT).T @ qT, so the
    softmax denominator comes from a ones-column appended to V during the
    PV matmul (O^T_ext = [V | 1]^T @ P^T), and P^T feeds the PV matmul
    directly without any transposes.
  - No max-subtraction in softmax: scores are ~N(0,1) by construction.
  - q-blocks are 512 wide so every matmul writes a single PSUM bank.
  - The S matmul runs two j-tiles ahead of the PV matmul (software
    pipeline) so the PE never stalls on the Exp activation.
  - The softmax reciprocal runs on the Scalar engine (activation table)
    straight out of PSUM; the [1, 512] DVE reciprocal it replaces was
    partition-serial and cost 6.5us per head.
  - Attention for q-block i and its projection are emitted right after
    QKV quarter i, overlapping everything with the remaining QKV work.
"""

import numpy as np

B, T, C = 4, 2048, 1024
H_PER_CORE = 8
D = 64
GC = 512  # channels per head-group (8 heads * 64)

_CACHE = {}


def _build_nc(t=T, reps=1, stages='all'):
    from contextlib import ExitStack

    import concourse.bacc as bacc
    import concourse.mybir as mybir
    import concourse.tile as tile

    fp32 = mybir.dt.float32
    bf16 = mybir.dt.bfloat16
    Exp = mybir.ActivationFunctionType.Exp
    Ln = mybir.ActivationFunctionType.Ln

    nt = t // 128          # token tiles
    nq = t // 512          # 512-token quarters / q-blocks

    nc = bacc.Bacc("TRN2", target_bir_lowering=False, debug=False, num_devices=8)

    xT_d = nc.dram_tensor("xT", [128, 8, t], bf16, kind="ExternalInput").ap()
    wqk_d = nc.dram_tensor("wqk", [128, 8, 1024], bf16, kind="ExternalInput").ap()
    wv_d = nc.dram_tensor("wv", [128, 8, GC], bf16, kind="ExternalInput").ap()
    wp_d = nc.dram_tensor("wp", [128, 4, 1024], bf16, kind="ExternalInput").ap()
    out_d = nc.dram_tensor("out", [t, C], fp32, kind="ExternalOutput").ap()

    with (
        tile.TileContext(nc) as tc,
        ExitStack() as top,
        nc.allow_low_precision(reason="bf16 tiles for PE-rate matmuls"),
    ):
        consts = top.enter_context(tc.tile_pool(name="consts", bufs=1))
        # gpsimd can't write bf16; build the mask via affine_select on a
        # bf16 tile after a memset through gpsimd's fp32 view is unsafe,
        # so keep the original recipe: memset+affine_select on bf16 works
        # through gpsimd ops that treat it as raw fill.
        mask01 = consts.tile([128, 128], bf16)
        nc.gpsimd.memset(mask01[:], 1.0)
        nc.gpsimd.affine_select(
            out=mask01[:], in_=mask01[:],
            compare_op=mybir.AluOpType.is_ge, fill=0.0, base=0,
            pattern=[[1, 128]], channel_multiplier=-1,
        )

        persist = top.enter_context(tc.tile_pool(name="persist", bufs=1))
        # q and k in [channel, T] (bf16): ptiles 0..3 = q (head h -> ptile
        # h//2, partitions (h%2)*64..), ptiles 4..7 = k
        qkT = persist.tile([128, 8, t], bf16)
        # v in [T, channel] + ones column (bf16)
        V = persist.tile([128, nt, 8, 65], bf16)

        nc.gpsimd.memset(V[:, :, :, 64:65], 1.0)

        for rep in range(reps):
            with ExitStack() as repstack:
                wpool = repstack.enter_context(
                    tc.tile_pool(name=f"w{rep}", bufs=1))
                xpool = repstack.enter_context(
                    tc.tile_pool(name=f"xT{rep}", bufs=2))
                apool = repstack.enter_context(
                    tc.tile_pool(name=f"attnwork{rep}", bufs=3))
                aopool = repstack.enter_context(
                    tc.tile_pool(name=f"ao{rep}", bufs=2))
                opool = repstack.enter_context(
                    tc.tile_pool(name=f"out{rep}", bufs=2))
                # PSUM: 3 + 3 + 2 = 8 banks
                spsum = repstack.enter_context(
                    tc.tile_pool(name=f"spsum{rep}", bufs=3, space="PSUM"))
                opsum = repstack.enter_context(
                    tc.tile_pool(name=f"opsum{rep}", bufs=3, space="PSUM"))
                gpsum = repstack.enter_context(
                    tc.tile_pool(name=f"gpsum{rep}", bufs=2, space="PSUM"))

                wqk_sb = wpool.tile([128, 8, 1024], bf16)
                wv_sb = wpool.tile([128, 8, GC], bf16)
                wp_sb = wpool.tile([128, 4, 1024], bf16)

                def emit_quarter(qtr):
                    xT_q = xpool.tile([128, 8, 512], bf16, tag="xTq",
                                      name=f"xT_q{rep}_{qtr}")
                    nc.sync.dma_start(
                        xT_q[:], xT_d[:, :, qtr * 512:(qtr + 1) * 512])
                    if qtr == 0:
                        # weight chunks stream in; first matmuls start early
                        for cc in range(8):
                            nc.sync.dma_start(
                                wqk_sb[:, cc, :], wqk_d[:, cc, :])
                            nc.sync.dma_start(
                                wv_sb[:, cc, :], wv_d[:, cc, :])
                        nc.sync.dma_start(wp_sb[:], wp_d[:])
                    for m in range(8):
                        ps_qk = gpsum.tile([128, 512], fp32, tag="gp",
                                           name=f"ps_qk{rep}_{qtr}_{m}")
                        for cc in range(8):
                            nc.tensor.matmul(
                                ps_qk[:],
                                wqk_sb[:, cc, m * 128:(m + 1) * 128],
                                xT_q[:, cc, :],
                                start=(cc == 0),
                                stop=(cc == 7),
                            )
                        nc.vector.tensor_copy(
                            qkT[:, m, qtr * 512:(qtr + 1) * 512], ps_qk[:])
                    for tt in range(4):
                        ttile = qtr * 4 + tt
                        ps_v = gpsum.tile([128, 512], fp32, tag="gp",
                                          name=f"ps_v{rep}_{qtr}_{tt}")
                        for cc in range(8):
                            nc.tensor.matmul(
                                ps_v[:],
                                xT_q[:, cc, tt * 128:(tt + 1) * 128],
                                wv_sb[:, cc, :],
                                start=(cc == 0),
                                stop=(cc == 7),
                            )
                        nc.vector.tensor_copy(
                            V[:, ttile, :, 0:64],
                            ps_v[:].rearrange("p (h d) -> p h d", h=8),
                        )

                def emit_attention(Q):
                    aoT_q = aopool.tile([128, 4, 512], bf16, tag="aoT",
                                        name=f"aoT_q{rep}_{Q}")
                    nj = 4 * Q + 4
                    for h in range(H_PER_CORE):
                        pbase = (h % 2) * 64
                        qT_h = qkT[pbase:pbase + 64, h // 2,
                                   Q * 512:(Q + 1) * 512]
                        kT_h = qkT[pbase:pbase + 64, 4 + h // 2, :]
                        ps_O = opsum.tile([65, 512], fp32, tag="ps_O",
                                          name=f"ps_O{rep}_{Q}_{h}")
                        PTs = {}

                        def emit_S(j):
                            off = max(0, (j - 4 * Q) * 128)
                            ps_S = spsum.tile([128, 512], fp32, tag="ps_S",
                                              name=f"ps_S{rep}_{Q}_{h}_{j}")
                            nc.tensor.matmul(
                                ps_S[:, off:512],
                                kT_h[:, j * 128:(j + 1) * 128],
                                qT_h[:, off:512],
                                start=True,
                                stop=True,
                            )
                            PT = apool.tile([128, 512], bf16, tag="PT",
                                            bufs=6,
                                            name=f"PT{rep}_{Q}_{h}_{j}")
                            nc.scalar.activation(
                                PT[:, off:512], ps_S[:, off:512],
                                Exp, scale=0.125,
                            )
                            if off > 0 or j == 4 * Q:
                                # diag tile: zero the k > q triangle post-exp
                                nc.gpsimd.tensor_mul(
                                    PT[:, off:off + 128],
                                    PT[:, off:off + 128],
                                    mask01[:],
                                )
                            PTs[j] = (PT, off)

                        def emit_PV(j):
                            PT, off = PTs.pop(j)
                            nc.tensor.matmul(
                                ps_O[:, off:512],
                                V[:, j, h, :],
                                PT[:, off:512],
                                start=(j == 0),
                                stop=(j == nj - 1),
                            )

                        # software pipeline: S runs 2 tiles ahead of PV
                        for j in range(nj):
                            emit_S(j)
                            if j >= 2:
                                emit_PV(j - 2)
                        emit_PV(nj - 2)
                        emit_PV(nj - 1)

                        # softmax normalize: 1/d = exp(-ln d) on the Scalar
                        # engine (its Reciprocal table is blocked for
                        # accuracy; Ln/Exp tables are fine), broadcast on
                        # gpsimd, multiply on DVE straight out of PSUM.
                        lnd = apool.tile([1, 512], fp32, tag="lnd",
                                         name=f"lnd{rep}_{Q}_{h}")
                        nc.scalar.activation(
                            lnd[:], ps_O[64:65, :], Ln)
                        rec = apool.tile([1, 512], fp32, tag="rec",
                                         name=f"rec{rep}_{Q}_{h}")
                        nc.scalar.activation(
                            rec[:], lnd[:], Exp, scale=-1.0)
                        rb = apool.tile([64, 512], fp32, tag="rb",
                                        name=f"rb{rep}_{Q}_{h}")
                        nc.gpsimd.partition_broadcast(rb[:], rec[:])
                        nc.vector.tensor_mul(
                            aoT_q[pbase:pbase + 64, h // 2, :],
                            ps_O[0:64, :],
                            rb[:],
                        )
                    return aoT_q

                def emit_proj(Q, aoT_q):
                    for tq in range(4):
                        ttile = Q * 4 + tq
                        out_sb = opool.tile([128, 1024], fp32, tag="out_sb",
                                            name=f"out_sb{rep}_{Q}_{tq}")
                        for hb in range(2):
                            ps_P = gpsum.tile([128, 512], fp32, tag="gp",
                                              name=f"ps_P{rep}_{Q}_{tq}_{hb}")
                            for cc in range(4):
                                nc.tensor.matmul(
                                    ps_P[:],
                                    aoT_q[:, cc, tq * 128:(tq + 1) * 128],
                                    wp_sb[:, cc, hb * 512:(hb + 1) * 512],
                                    start=(cc == 0),
                                    stop=(cc == 3),
                                )
                            nc.vector.tensor_copy(
                                out_sb[:, hb * 512:(hb + 1) * 512], ps_P[:])
                        nc.sync.dma_start(
                            out_d[ttile * 128:(ttile + 1) * 128, :], out_sb[:])

                for qtr in range(nq):
                    emit_quarter(qtr)
                    if stages != 'all':
                        continue
                    aoT_q = emit_attention(qtr)
                    emit_proj(qtr, aoT_q)

    nc.compile()
    return nc


def _get_nc(t=T, reps=1, stages='all'):
    key = (t, reps, stages)
    if key not in _CACHE:
        _CACHE[key] = _build_nc(t, reps, stages)
    return _CACHE[key]


def _bf16(a):
    import ml_dtypes
    return np.ascontiguousarray(a.astype(ml_dtypes.bfloat16))


def _pack_weights(w_qkv, w_proj, g):
    """Per-group weight slices, pre-arranged into the SBUF tile layouts."""
    wq = w_qkv[GC * g:GC * (g + 1), :]
    wk = w_qkv[C + GC * g:C + GC * (g + 1), :]
    wv = w_qkv[2 * C + GC * g:2 * C + GC * (g + 1), :]
    wqkT = np.ascontiguousarray(np.concatenate([wq, wk], axis=0).T)  # [C, 1024]
    wqk_arr = np.ascontiguousarray(
        wqkT.reshape(8, 128, 1024).transpose(1, 0, 2))
    wvT = np.ascontiguousarray(wv.T)  # [C, 512]
    wv_arr = np.ascontiguousarray(wvT.reshape(8, 128, GC).transpose(1, 0, 2))
    wpT = np.ascontiguousarray(w_proj[:, GC * g:GC * (g + 1)].T)  # [512, 1024]
    wp_arr = np.ascontiguousarray(wpT.reshape(4, 128, 1024).transpose(1, 0, 2))
    return _bf16(wqk_arr), _bf16(wv_arr), _bf16(wp_arr)


def _get_runner():
    """Build (once) a cached sharded-jit runner for the 8-core NEFF.

    Mirrors concourse.bass2jax.run_bass_via_pjrt's multi-core path, but
    caches the jit callable and the device-resident zero output buffers
    so repeat calls only pay input transfer + execution.
    """
    if "runner" in _CACHE:
        return _CACHE["runner"]

    import jax
    from jax.experimental.shard_map import shard_map
    from jax.sharding import Mesh, PartitionSpec

    import concourse.mybir as mybir
    from concourse.bass2jax import (
        _bass_exec_p,
        install_neuronx_cc_hook,
        partition_id_tensor,
    )

    install_neuronx_cc_hook()
    nc = _get_nc()
    n_cores = 8

    in_names, out_names, out_avals = [], [], []
    partition_name = (
        nc.partition_id_tensor.name if nc.partition_id_tensor else None
    )
    for alloc in nc.m.functions[0].allocations:
        if not isinstance(alloc, mybir.MemoryLocationSet):
            continue
        name = alloc.memorylocations[0].name
        if alloc.kind == "ExternalInput":
            if name != partition_name:
                in_names.append(name)
        elif alloc.kind == "ExternalOutput":
            out_names.append(name)
            out_avals.append(
                jax.core.ShapedArray(
                    tuple(alloc.tensor_shape), mybir.dt.np(alloc.dtype)
                )
            )
    n_params = len(in_names)
    all_in_names = in_names + out_names
    if partition_name is not None:
        all_in_names.append(partition_name)

    def _body(*args):
        operands = list(args)
        if partition_name is not None:
            operands.append(partition_id_tensor())
        outs = _bass_exec_p.bind(
            *operands,
            out_avals=tuple(out_avals),
            in_names=tuple(all_in_names),
            out_names=tuple(out_names),
            lowering_input_output_aliases=(),
            sim_require_finite=True,
            sim_require_nnan=True,
            nc=nc,
        )
        return tuple(outs)

    devices = jax.devices()[:n_cores]
    mesh = Mesh(np.asarray(devices), ("core",))
    in_specs = (PartitionSpec("core"),) * (n_params + len(out_names))
    out_specs = (PartitionSpec("core"),) * len(out_names)
    fn = jax.jit(
        shard_map(_body, mesh=mesh, in_specs=in_specs,
                  out_specs=out_specs, check_rep=False),
        keep_unused=True,
    )
    zero_sharding = jax.sharding.NamedSharding(mesh, PartitionSpec("core"))
    dev_zeros = [
        jax.device_put(
            np.zeros((n_cores * av.shape[0], *av.shape[1:]), av.dtype),
            zero_sharding,
        )
        for av in out_avals
    ]
    runner = {
        "fn": fn,
        "in_names": in_names,
        "out_names": out_names,
        "out_avals": out_avals,
        "dev_zeros": dev_zeros,
        "sharding": zero_sharding,
        "n_cores": n_cores,
    }
    _CACHE["runner"] = runner
    return runner


def _make_in_maps(x, w_qkv, w_proj):
    x = np.ascontiguousarray(np.asarray(x, dtype=np.float32))
    w_qkv = np.ascontiguousarray(np.asarray(w_qkv, dtype=np.float32))
    w_proj = np.ascontiguousarray(np.asarray(w_proj, dtype=np.float32))
    packed = [_pack_weights(w_qkv, w_proj, g) for g in range(2)]
    in_maps = []
    for core in range(8):
        b, g = core // 2, core % 2
        wqk_arr, wv_arr, wp_arr = packed[g]
        xT = np.ascontiguousarray(x[b].T)  # [C, T]
        xT_arr = np.ascontiguousarray(
            xT.reshape(8, 128, T).transpose(1, 0, 2))
        in_maps.append({
            "xT": _bf16(xT_arr),
            "wqk": wqk_arr,
            "wv": wv_arr,
            "wp": wp_arr,
        })
    return in_maps


def _device_inputs(runner, in_maps):
    import jax

    concat = [
        np.concatenate([in_maps[c][name] for c in range(runner["n_cores"])],
                       axis=0)
        for name in runner["in_names"]
    ]
    return [jax.device_put(a, runner["sharding"]) for a in concat]


def _exec(runner, dev_in):
    return runner["fn"](*dev_in, *runner["dev_zeros"])


def _run(x, w_qkv, w_proj, b_proj):
    b_proj = np.asarray(b_proj, dtype=np.float32)
    runner = _get_runner()
    in_maps = _make_in_maps(x, w_qkv, w_proj)
    dev_in = _device_inputs(runner, in_maps)
    out_arrs = _exec(runner, dev_in)
    parts = np.asarray(out_arrs[0]).reshape(8, T, C)
    out = np.empty((B, T, C), dtype=np.float32)
    for b in range(B):
        out[b] = parts[2 * b] + parts[2 * b + 1]
    out += b_proj
    return out, None


def kernel(x, w_qkv, w_proj, b_proj):
    out, _ = _run(x, w_qkv, w_proj, b_proj)
    return out


# revision 14
# speedup vs baseline: 1.1687x; 1.1687x over previous
"""Causal self-attention (B=4, T=2048, C=1024, H=16) on 8 TRN2 NeuronCores.

Sharding: core = (batch b, head-group g); 4 batches x 2 groups of 8 heads.
Each core computes QKV for its 8 heads on its batch, causal attention, and
a partial projection output [T, C] (sum over its heads' channels). The host
sums the two group-partials per batch and adds b_proj.

v2 layout choices (per core):
  - x arrives pre-transposed from the host as xT [C, T] so the QKV matmuls
    contract along the partition dim with no on-chip transposes.
  - qT/kT are produced in [channel, T] layout, v in [T, channel] layout.
  - Scores are computed transposed: S^T[k, q] = lhsT(kT).T @ qT, so the
    softmax denominator comes from a ones-column appended to V during the
    PV matmul (O^T_ext = [V | 1]^T @ P^T), and P^T feeds the PV matmul
    directly without any transposes.
  - No max-subtraction in softmax: scores are ~N(0,1) by construction.
  - q-blocks are 512 wide so every matmul writes a single PSUM bank.
  - The S matmul runs two j-tiles ahead of the PV matmul (software
    pipeline) so the PE never stalls on the Exp activation.
  - The softmax reciprocal runs on the Scalar engine (activation table)
    straight out of PSUM; the [1, 512] DVE reciprocal it replaces was
    partition-serial and cost 6.5us per head.
  - Attention for q-block i and its projection are emitted right after
    QKV quarter i, overlapping everything with the remaining QKV work.
"""

import numpy as np

B, T, C = 4, 2048, 1024
H_PER_CORE = 8
D = 64
GC = 512  # channels per head-group (8 heads * 64)

_CACHE = {}


def _build_nc(t=T, reps=1, stages='all'):
    from contextlib import ExitStack

    import concourse.bacc as bacc
    import concourse.mybir as mybir
    import concourse.tile as tile

    fp32 = mybir.dt.float32
    bf16 = mybir.dt.bfloat16
    Exp = mybir.ActivationFunctionType.Exp

    nt = t // 128          # token tiles
    nq = t // 512          # 512-token quarters / q-blocks

    nc = bacc.Bacc("TRN2", target_bir_lowering=False, debug=False, num_devices=8)

    xT_d = nc.dram_tensor("xT", [128, 8, t], bf16, kind="ExternalInput").ap()
    wqk_d = nc.dram_tensor("wqk", [128, 8, 1024], bf16, kind="ExternalInput").ap()
    wv_d = nc.dram_tensor("wv", [128, 8, GC], bf16, kind="ExternalInput").ap()
    wp_d = nc.dram_tensor("wp", [128, 4, 1024], bf16, kind="ExternalInput").ap()
    out_d = nc.dram_tensor("out", [t, C], fp32, kind="ExternalOutput").ap()

    with (
        tile.TileContext(nc) as tc,
        ExitStack() as top,
        nc.allow_low_precision(reason="bf16 tiles for PE-rate matmuls"),
    ):
        consts = top.enter_context(tc.tile_pool(name="consts", bufs=1))
        # gpsimd can't write bf16; build the mask via affine_select on a
        # bf16 tile after a memset through gpsimd's fp32 view is unsafe,
        # so keep the original recipe: memset+affine_select on bf16 works
        # through gpsimd ops that treat it as raw fill.
        mask01 = consts.tile([128, 128], bf16)
        nc.gpsimd.memset(mask01[:], 1.0)
        nc.gpsimd.affine_select(
            out=mask01[:], in_=mask01[:],
            compare_op=mybir.AluOpType.is_ge, fill=0.0, base=0,
            pattern=[[1, 128]], channel_multiplier=-1,
        )

        persist = top.enter_context(tc.tile_pool(name="persist", bufs=1))
        # q and k in [channel, T] (bf16): ptiles 0..3 = q (head h -> ptile
        # h//2, partitions (h%2)*64..), ptiles 4..7 = k
        qkT = persist.tile([128, 8, t], bf16)
        # v in [T, channel] + ones column (bf16)
        V = persist.tile([128, nt, 8, 65], bf16)

        nc.gpsimd.memset(V[:, :, :, 64:65], 1.0)

        for rep in range(reps):
            with ExitStack() as repstack:
                wpool = repstack.enter_context(
                    tc.tile_pool(name=f"w{rep}", bufs=1))
                xpool = repstack.enter_context(
                    tc.tile_pool(name=f"xT{rep}", bufs=2))
                apool = repstack.enter_context(
                    tc.tile_pool(name=f"attnwork{rep}", bufs=3))
                aopool = repstack.enter_context(
                    tc.tile_pool(name=f"ao{rep}", bufs=2))
                opool = repstack.enter_context(
                    tc.tile_pool(name=f"out{rep}", bufs=2))
                # PSUM: 3 + 3 + 2 = 8 banks
                spsum = repstack.enter_context(
                    tc.tile_pool(name=f"spsum{rep}", bufs=3, space="PSUM"))
                opsum = repstack.enter_context(
                    tc.tile_pool(name=f"opsum{rep}", bufs=3, space="PSUM"))
                gpsum = repstack.enter_context(
                    tc.tile_pool(name=f"gpsum{rep}", bufs=2, space="PSUM"))

                wqk_sb = wpool.tile([128, 8, 1024], bf16)
                wv_sb = wpool.tile([128, 8, GC], bf16)
                wp_sb = wpool.tile([128, 4, 1024], bf16)

                def emit_quarter(qtr):
                    xT_q = xpool.tile([128, 8, 512], bf16, tag="xTq",
                                      name=f"xT_q{rep}_{qtr}")
                    nc.sync.dma_start(
                        xT_q[:], xT_d[:, :, qtr * 512:(qtr + 1) * 512])
                    if qtr == 0:
                        # weight chunks stream in; first matmuls start early
                        for cc in range(8):
                            nc.sync.dma_start(
                                wqk_sb[:, cc, :], wqk_d[:, cc, :])
                            nc.sync.dma_start(
                                wv_sb[:, cc, :], wv_d[:, cc, :])
                        nc.sync.dma_start(wp_sb[:], wp_d[:])
                    for m in range(8):
                        ps_qk = gpsum.tile([128, 512], fp32, tag="gp",
                                           name=f"ps_qk{rep}_{qtr}_{m}")
                        for cc in range(8):
                            nc.tensor.matmul(
                                ps_qk[:],
                                wqk_sb[:, cc, m * 128:(m + 1) * 128],
                                xT_q[:, cc, :],
                                start=(cc == 0),
                                stop=(cc == 7),
                            )
                        nc.vector.tensor_copy(
                            qkT[:, m, qtr * 512:(qtr + 1) * 512], ps_qk[:])
                    for tt in range(4):
                        ttile = qtr * 4 + tt
                        ps_v = gpsum.tile([128, 512], fp32, tag="gp",
                                          name=f"ps_v{rep}_{qtr}_{tt}")
                        for cc in range(8):
                            nc.tensor.matmul(
                                ps_v[:],
                                xT_q[:, cc, tt * 128:(tt + 1) * 128],
                                wv_sb[:, cc, :],
                                start=(cc == 0),
                                stop=(cc == 7),
                            )
                        nc.vector.tensor_copy(
                            V[:, ttile, :, 0:64],
                            ps_v[:].rearrange("p (h d) -> p h d", h=8),
                        )

                def emit_attention(Q):
                    aoT_q = aopool.tile([128, 4, 512], bf16, tag="aoT",
                                        name=f"aoT_q{rep}_{Q}")
                    # unnormalized O^T for all 8 heads + per-head reciprocal
                    # rows; the multiply happens per head-pair in
                    # emit_normalize, hidden under the next QKV quarter.
                    ocp = aopool.tile([128, 4, 512], fp32, tag="ocp",
                                      name=f"ocp{rep}_{Q}")
                    recs = []
                    nj = 4 * Q + 4
                    for h in range(H_PER_CORE):
                        pbase = (h % 2) * 64
                        qT_h = qkT[pbase:pbase + 64, h // 2,
                                   Q * 512:(Q + 1) * 512]
                        kT_h = qkT[pbase:pbase + 64, 4 + h // 2, :]
                        ps_O = opsum.tile([65, 512], fp32, tag="ps_O",
                                          name=f"ps_O{rep}_{Q}_{h}")
                        PTs = {}

                        def emit_S(j):
                            off = max(0, (j - 4 * Q) * 128)
                            ps_S = spsum.tile([128, 512], fp32, tag="ps_S",
                                              name=f"ps_S{rep}_{Q}_{h}_{j}")
                            nc.tensor.matmul(
                                ps_S[:, off:512],
                                kT_h[:, j * 128:(j + 1) * 128],
                                qT_h[:, off:512],
                                start=True,
                                stop=True,
                            )
                            PT = apool.tile([128, 512], bf16, tag="PT",
                                            bufs=6,
                                            name=f"PT{rep}_{Q}_{h}_{j}")
                            nc.scalar.activation(
                                PT[:, off:512], ps_S[:, off:512],
                                Exp, scale=0.125,
                            )
                            if off > 0 or j == 4 * Q:
                                # diag tile: zero the k > q triangle post-exp
                                nc.gpsimd.tensor_mul(
                                    PT[:, off:off + 128],
                                    PT[:, off:off + 128],
                                    mask01[:],
                                )
                            PTs[j] = (PT, off)

                        def emit_PV(j):
                            PT, off = PTs.pop(j)
                            nc.tensor.matmul(
                                ps_O[:, off:512],
                                V[:, j, h, :],
                                PT[:, off:512],
                                start=(j == 0),
                                stop=(j == nj - 1),
                            )

                        # software pipeline: S runs 2 tiles ahead of PV
                        for j in range(nj):
                            emit_S(j)
                            if j >= 2:
                                emit_PV(j - 2)
                        emit_PV(nj - 2)
                        emit_PV(nj - 1)

                        # drain PSUM promptly: unnormalized O^T to SBUF and
                        # the per-head reciprocal of the ones-row denominator
                        nc.vector.tensor_copy(
                            ocp[pbase:pbase + 64, h // 2, :], ps_O[0:64, :])
                        rec = apool.tile([1, 512], fp32, tag="rec", bufs=18,
                                         name=f"rec{rep}_{Q}_{h}")
                        nc.vector.reciprocal(rec[:], ps_O[64:65, :])
                        recs.append(rec)
                    return aoT_q, ocp, recs

                def emit_normalize(Q, aoT_q, ocp, recs):
                    for h in range(H_PER_CORE):
                        pbase = (h % 2) * 64
                        rb = apool.tile([128, 512], fp32, tag="rb",
                                        name=f"rb{rep}_{Q}_{h}")
                        nc.gpsimd.partition_broadcast(rb[:], recs[h][:])
                        nc.vector.tensor_mul(
                            aoT_q[pbase:pbase + 64, h // 2, :],
                            ocp[pbase:pbase + 64, h // 2, :],
                            rb[pbase:pbase + 64, :])

                def emit_proj(Q, aoT_q):
                    for tq in range(4):
                        ttile = Q * 4 + tq
                        out_sb = opool.tile([128, 1024], fp32, tag="out_sb",
                                            name=f"out_sb{rep}_{Q}_{tq}")
                        for hb in range(2):
                            ps_P = gpsum.tile([128, 512], fp32, tag="gp",
                                              name=f"ps_P{rep}_{Q}_{tq}_{hb}")
                            for cc in range(4):
                                nc.tensor.matmul(
                                    ps_P[:],
                                    aoT_q[:, cc, tq * 128:(tq + 1) * 128],
                                    wp_sb[:, cc, hb * 512:(hb + 1) * 512],
                                    start=(cc == 0),
                                    stop=(cc == 3),
                                )
                            nc.vector.tensor_copy(
                                out_sb[:, hb * 512:(hb + 1) * 512], ps_P[:])
                        nc.sync.dma_start(
                            out_d[ttile * 128:(ttile + 1) * 128, :], out_sb[:])

                # order: q0 a0 | n0 q1 p0 a1 | n1 q2 p1 a2 | n2 q3 p2 a3
                # | n3 p3 -- block i's softmax normalization (DVE/gpsimd)
                # runs under quarter i+1's QKV matmuls, so proj(i) finds
                # aoT ready and the PE never stalls on the tail chain.
                pend = None
                for qtr in range(nq):
                    if pend is not None:
                        emit_normalize(*pend)
                    emit_quarter(qtr)
                    if stages != 'all':
                        continue
                    if pend is not None:
                        emit_proj(pend[0], pend[1])
                    pend = (qtr,) + emit_attention(qtr)
                if pend is not None:
                    emit_normalize(*pend)
                    emit_proj(pend[0], pend[1])

    nc.compile()
    return nc


def _get_nc(t=T, reps=1, stages='all'):
    key = (t, reps, stages)
    if key not in _CACHE:
        _CACHE[key] = _build_nc(t, reps, stages)
    return _CACHE[key]


def _bf16(a):
    import ml_dtypes
    return np.ascontiguousarray(a.astype(ml_dtypes.bfloat16))


def _pack_weights(w_qkv, w_proj, g):
    """Per-group weight slices, pre-arranged into the SBUF tile layouts."""
    wq = w_qkv[GC * g:GC * (g + 1), :]
    wk = w_qkv[C + GC * g:C + GC * (g + 1), :]
    wv = w_qkv[2 * C + GC * g:2 * C + GC * (g + 1), :]
    wqkT = np.ascontiguousarray(np.concatenate([wq, wk], axis=0).T)  # [C, 1024]
    wqk_arr = np.ascontiguousarray(
        wqkT.reshape(8, 128, 1024).transpose(1, 0, 2))
    wvT = np.ascontiguousarray(wv.T)  # [C, 512]
    wv_arr = np.ascontiguousarray(wvT.reshape(8, 128, GC).transpose(1, 0, 2))
    wpT = np.ascontiguousarray(w_proj[:, GC * g:GC * (g + 1)].T)  # [512, 1024]
    wp_arr = np.ascontiguousarray(wpT.reshape(4, 128, 1024).transpose(1, 0, 2))
    return _bf16(wqk_arr), _bf16(wv_arr), _bf16(wp_arr)


def _get_runner():
    """Build (once) a cached sharded-jit runner for the 8-core NEFF.

    Mirrors concourse.bass2jax.run_bass_via_pjrt's multi-core path, but
    caches the jit callable and the device-resident zero output buffers
    so repeat calls only pay input transfer + execution.
    """
    if "runner" in _CACHE:
        return _CACHE["runner"]

    import jax
    from jax.experimental.shard_map import shard_map
    from jax.sharding import Mesh, PartitionSpec

    import concourse.mybir as mybir
    from concourse.bass2jax import (
        _bass_exec_p,
        install_neuronx_cc_hook,
        partition_id_tensor,
    )

    install_neuronx_cc_hook()
    nc = _get_nc()
    n_cores = 8

    in_names, out_names, out_avals = [], [], []
    partition_name = (
        nc.partition_id_tensor.name if nc.partition_id_tensor else None
    )
    for alloc in nc.m.functions[0].allocations:
        if not isinstance(alloc, mybir.MemoryLocationSet):
            continue
        name = alloc.memorylocations[0].name
        if alloc.kind == "ExternalInput":
            if name != partition_name:
                in_names.append(name)
        elif alloc.kind == "ExternalOutput":
            out_names.append(name)
            out_avals.append(
                jax.core.ShapedArray(
                    tuple(alloc.tensor_shape), mybir.dt.np(alloc.dtype)
                )
            )
    n_params = len(in_names)
    all_in_names = in_names + out_names
    if partition_name is not None:
        all_in_names.append(partition_name)

    def _body(*args):
        operands = list(args)
        if partition_name is not None:
            operands.append(partition_id_tensor())
        outs = _bass_exec_p.bind(
            *operands,
            out_avals=tuple(out_avals),
            in_names=tuple(all_in_names),
            out_names=tuple(out_names),
            lowering_input_output_aliases=(),
            sim_require_finite=True,
            sim_require_nnan=True,
            nc=nc,
        )
        return tuple(outs)

    devices = jax.devices()[:n_cores]
    mesh = Mesh(np.asarray(devices), ("core",))
    in_specs = (PartitionSpec("core"),) * (n_params + len(out_names))
    out_specs = (PartitionSpec("core"),) * len(out_names)
    fn = jax.jit(
        shard_map(_body, mesh=mesh, in_specs=in_specs,
                  out_specs=out_specs, check_rep=False),
        keep_unused=True,
    )
    zero_sharding = jax.sharding.NamedSharding(mesh, PartitionSpec("core"))
    dev_zeros = [
        jax.device_put(
            np.zeros((n_cores * av.shape[0], *av.shape[1:]), av.dtype),
            zero_sharding,
        )
        for av in out_avals
    ]
    runner = {
        "fn": fn,
        "in_names": in_names,
        "out_names": out_names,
        "out_avals": out_avals,
        "dev_zeros": dev_zeros,
        "sharding": zero_sharding,
        "n_cores": n_cores,
    }
    _CACHE["runner"] = runner
    return runner


def _make_in_maps(x, w_qkv, w_proj):
    x = np.ascontiguousarray(np.asarray(x, dtype=np.float32))
    w_qkv = np.ascontiguousarray(np.asarray(w_qkv, dtype=np.float32))
    w_proj = np.ascontiguousarray(np.asarray(w_proj, dtype=np.float32))
    packed = [_pack_weights(w_qkv, w_proj, g) for g in range(2)]
    in_maps = []
    for core in range(8):
        b, g = core // 2, core % 2
        wqk_arr, wv_arr, wp_arr = packed[g]
        xT = np.ascontiguousarray(x[b].T)  # [C, T]
        xT_arr = np.ascontiguousarray(
            xT.reshape(8, 128, T).transpose(1, 0, 2))
        in_maps.append({
            "xT": _bf16(xT_arr),
            "wqk": wqk_arr,
            "wv": wv_arr,
            "wp": wp_arr,
        })
    return in_maps


def _device_inputs(runner, in_maps):
    import jax

    concat = [
        np.concatenate([in_maps[c][name] for c in range(runner["n_cores"])],
                       axis=0)
        for name in runner["in_names"]
    ]
    return [jax.device_put(a, runner["sharding"]) for a in concat]


def _exec(runner, dev_in):
    return runner["fn"](*dev_in, *runner["dev_zeros"])


def _run(x, w_qkv, w_proj, b_proj):
    b_proj = np.asarray(b_proj, dtype=np.float32)
    runner = _get_runner()
    in_maps = _make_in_maps(x, w_qkv, w_proj)
    dev_in = _device_inputs(runner, in_maps)
    out_arrs = _exec(runner, dev_in)
    parts = np.asarray(out_arrs[0]).reshape(8, T, C)
    out = np.empty((B, T, C), dtype=np.float32)
    for b in range(B):
        out[b] = parts[2 * b] + parts[2 * b + 1]
    out += b_proj
    return out, None


def kernel(x, w_qkv, w_proj, b_proj):
    out, _ = _run(x, w_qkv, w_proj, b_proj)
    return out
